# revision 67
# baseline (speedup 1.0000x reference)
"""Trainium2 Bass kernel for nn_AttentionLayer (additive attention pooling).

reference math:
    re = entities @ w1_w + w1_b                  # [B, H]
    rc = contexts @ w2_w + w2_b                  # [B, S, H]
    scores = tanh(re[:,None,:] + rc) @ v_w + v_b # [B, S, 1]
    weights = softmax(scores, axis=1)
    out = weights * contexts                     # [B, S, D]

Sharding: data-parallel over B across 8 cores (4 batches/core), weights
replicated.  Inside each core: bf16 TensorEngine matmuls (f32 accumulate),
softmax in f32/f16.  v_b is dropped (softmax is shift-invariant).

Dataflow (per core), engineered so the TensorE 216ns/matmul stream never
stalls:
  - dual-ring preamble: w1 streams on the scalar HWDGE ring (feeding the
    entity path at the PE-program head, its last two quarters deferred into
    chunk-0 slots), contexts-c0 + w2 + remaining batch-0 contexts on the
    sync ring; casts on DVE ordered by consumption deadline.
  - per 512-token chunk: one xbar DMA transpose (sync ring) makes the
    d-major bf16 rhs; 64 accumulating bf16 matmuls produce rc; tanh(+re
    bias) on ACT; the 8 v-matvecs of a chunk run as one consecutive block
    one chunk later (entering/leaving the matvec costs a ~190ns LDW-pipeline
    bubble, and the stagger hides the tanh latency).
  - sync-ring FIFO discipline: all of a batch's transposes are queued ahead
    of the next batch's context loads so a stalled context DMA can never
    head-of-line-block a deadline-critical transpose.
  - chunk scores [1,512] are copied to fp16 and transposed token-major via
    4 tiny PE matmuls into a per-batch [128,16] PSUM tile; softmax runs
    full-width (exp+accum on ACT, cross-partition total broadcast via a
    ones-matmul, reciprocal on DVE) - no single-lane work.
  - out tiles = bf16 contexts * per-token weight (DVE multiply), stored on
    the otherwise-idle SWDGE (gpsimd) ring, spread across the next batch's
    chunk slots; each prior batch's tail (matvec block, score transpose,
    softmax, stage F) rides the next batch's first-chunk slots.

(An fp8e4m3 DoubleRow variant of the main matmul was built and passed
at rel-err 9.7e-3, but on this stack DoubleRow streams the two fp8 planes
sequentially - no throughput gain over bf16 - so bf16 is kept.)
"""

import sys

for _p in ("/opt/trn_rl_repo", "/root/.axon_site/_ro/trn_rl_repo"):
    if _p not in sys.path:
        sys.path.insert(0, _p)

import numpy as np

B, S, D, H = 32, 2048, 1024, 1024
N_CORES = 8
B_LOC = B // N_CORES          # batches per core
P = 128
TCHUNK = 512                  # tokens per chunk (moving free dim of main matmul)


def build_attention(tc, out_ap, ins, b_loc=B_LOC, s=S, d=D, h=H):
    """Emit the per-core kernel into TileContext `tc`.

    out_ap: DRAM AP [b_loc*s, d] f32
    ins: dict of DRAM APs: contexts [b_loc*s, d], entities [b_loc, d],
         w1_w [d, h], w2_w [d, h], w1_b [h], w2_b [h], v_w [h, 1]
    """
    from contextlib import ExitStack

    import concourse.mybir as mybir
    from concourse.masks import make_identity

    nc = tc.nc
    f32 = mybir.dt.float32
    bf16 = mybir.dt.bfloat16
    f16 = mybir.dt.float16
    AF = mybir.ActivationFunctionType

    KO = d // P                   # contraction k-tiles
    HO = h // P                   # h tiles
    NT = s // P                   # 128-token tiles per batch
    NC = s // TCHUNK              # chunks per batch
    TPC = TCHUNK // P             # token tiles per chunk
    QW = 256                      # h-chunk width for weight staging DMAs
    NQ = h // QW
    EP = 32                       # padded partition count for entity transposes
    assert d % P == 0 and h % P == 0 and s % TCHUNK == 0 and b_loc <= EP

    ctx3 = ins["contexts"].rearrange("(n p) dd -> n p dd", p=P)   # [b_loc*NT, P, d]
    out3 = out_ap.rearrange("(n p) dd -> n p dd", p=P)
    w1_3d = ins["w1_w"].rearrange("(ko p) hh -> p ko hh", p=P)
    w2_3d = ins["w2_w"].rearrange("(ko p) hh -> p ko hh", p=P)

    with ExitStack() as ctx:
        consts = ctx.enter_context(tc.tile_pool(name="consts", bufs=1))
        wpool = ctx.enter_context(tc.tile_pool(name="wpool", bufs=1))

        # ---------------- constants (tiny, sync ring) ----------------
        id32 = consts.tile([EP, EP], f32, tag="id32")
        make_identity(nc, id32)
        ones4_f16 = consts.tile([b_loc, 1], f16, tag="ones4")
        nc.vector.memset(ones4_f16, 1.0)
        ones128_f16 = consts.tile([P, P], f16, tag="ones128")
        nc.vector.memset(ones128_f16, 1.0)

        ent_sb = consts.tile([EP, d], f32, tag="ent")
        nc.vector.memset(ent_sb, 0.0)
        nc.sync.dma_start(out=ent_sb[:b_loc, :], in_=ins["entities"][:, :])

        # biases and v load as contiguous [HO, P] rows (8 fat descriptors
        # instead of 1024 4-byte ones that would stall the ring for ~10us),
        # then are transposed to [P, HO] on the PE like the entity path
        b1_row = consts.tile([EP, P], f32, tag="b1r")
        b2_row = consts.tile([EP, P], f32, tag="b2r")
        v_row = consts.tile([EP, P], f32, tag="v_r")
        nc.vector.memset(b1_row, 0.0)
        nc.vector.memset(b2_row, 0.0)
        nc.vector.memset(v_row, 0.0)
        nc.sync.dma_start(
            out=b1_row[:HO, :], in_=ins["w1_b"].rearrange("(ho p) -> ho p", p=P)
        )
        nc.sync.dma_start(
            out=b2_row[:HO, :], in_=ins["w2_b"].rearrange("(ho p) -> ho p", p=P)
        )
        nc.sync.dma_start(
            out=v_row[:HO, :], in_=ins["v_w"].rearrange("(ho p) o -> ho (p o)", p=P)
        )
        bias_sb = consts.tile([P, HO], f32, tag="bias")

        # ---------------- weight / context tiles ----------------
        w1_bf = wpool.tile([P, KO, h], bf16, tag="w1bf")
        w2_bf = wpool.tile([P, KO, h], bf16, tag="w2bf")
        v_bf = consts.tile([P, HO], bf16, tag="v_bf")

        xbf_pool = ctx.enter_context(tc.tile_pool(name="xbf", bufs=8))
        xt_pool = ctx.enter_context(tc.tile_pool(name="xt", bufs=5))
        cin_pool = ctx.enter_context(tc.tile_pool(name="cin", bufs=3))
        # wst_pool is scoped: its SBUF is released before th/out/sm pools are
        # created, so the weight staging can afford 4 rotation buffers
        wst_ctx = ExitStack()
        wst_pool = wst_ctx.enter_context(tc.tile_pool(name="wst", bufs=4))

        wst_tiles = {}

        def load_w_dma(dst3, src3, q, eng):
            # HWDGE f32 load into a staging buffer; cast emitted separately
            # so the DMA order and cast order can differ
            wst = wst_pool.tile([P, KO, QW], f32, tag="wst")
            eng.dma_start(out=wst, in_=src3[:, :, q * QW : (q + 1) * QW])
            wst_tiles[(id(dst3), q)] = wst

        def cast_w(dst3, q):
            wst = wst_tiles.pop((id(dst3), q))
            nc.vector.tensor_copy(out=dst3[:, :, q * QW : (q + 1) * QW], in_=wst)

        xbf_tiles = {}
        xt_tiles = {}

        def load_ctx(b, c, eng=None):
            # per-chunk bf16 context tile, staged through two f32 half-chunk DMAs
            eng = eng or nc.sync
            xc = xbf_pool.tile([P, TPC, d], bf16, tag="xbf")
            r0 = b * NT + c * TPC
            for hf in range(2):
                cin = cin_pool.tile([P, 2, d], f32, tag="cin")
                eng.dma_start(
                    out=cin,
                    in_=ctx3[r0 + 2 * hf : r0 + 2 * hf + 2].rearrange(
                        "n p dd -> p n dd"
                    ),
                )
                nc.vector.tensor_copy(out=xc[:, 2 * hf : 2 * hf + 2, :], in_=cin)
            xbf_tiles[(b, c)] = xc

        def emit_transpose(b, T):
            xt = xt_pool.tile([P, TPC, KO, P], bf16, tag="xt", name="xt")
            nc.sync.dma_start(out=xt, in_=xbf_tiles[(b, T)], transpose=True)
            xt_tiles[(b, T)] = xt

        # ---------------- main-loop PSUM pools ----------------
        ps_rc = ctx.enter_context(tc.tile_pool(name="ps_rc", bufs=4, space="PSUM"))
        ps_sc = ctx.enter_context(tc.tile_pool(name="ps_sc", bufs=2, space="PSUM"))
        ps_wt = ctx.enter_context(tc.tile_pool(name="ps_wt", bufs=2, space="PSUM"))

        # ---------------- entity path: reb[:, ho, b] = (entities@w1 + b1+b2)^T ----
        reb_sb = consts.tile([P, HO, b_loc], f32, tag="reb")
        entT_bf = consts.tile([P, KO, b_loc], bf16, tag="entT")
        re_sb = consts.tile([EP, h], f32, tag="re_sb")

        # the tiny entity-path PSUM tiles borrow rc/sc bank slots (PSUM is
        # bank-granular and all 8 banks are budgeted for the main loop)
        def emit_ent_transposes():
            # ent transposes first: they only need ent_sb+id32 (~ready at
            # 10us); the bias/v transposes depend on DVE work queued behind
            # the big weight/context casts and would head-of-line-block the
            # PE if emitted first
            for ko in range(KO):
                etr = ps_rc.tile([P, TCHUNK], f32, tag="rc", name="etr")
                nc.tensor.transpose(
                    etr[:, :EP], ent_sb[:, ko * P : (ko + 1) * P], id32
                )
                nc.vector.tensor_copy(out=entT_bf[:, ko, :], in_=etr[:, :b_loc])
            badd_row = consts.tile([EP, P], f32, tag="badd")
            nc.vector.tensor_add(out=badd_row, in0=b1_row, in1=b2_row)
            btr = ps_rc.tile([P, TCHUNK], f32, tag="rc", name="btr")
            nc.tensor.transpose(btr[:, :EP], badd_row, id32)
            nc.vector.tensor_copy(out=bias_sb, in_=btr[:, :HO])
            vtr = ps_rc.tile([P, TCHUNK], f32, tag="rc", name="vtr")
            nc.tensor.transpose(vtr[:, :EP], v_row, id32)
            nc.vector.tensor_copy(out=v_bf, in_=vtr[:, :HO])
            nc.vector.memset(re_sb, 0.0)

        def emit_re_q(q):
            re_ps = ps_sc.tile([b_loc, TCHUNK], f32, tag="sc", name="re_ps")
            for ko in range(KO):
                nc.tensor.matmul(
                    re_ps[:, :QW],
                    lhsT=entT_bf[:, ko, :],
                    rhs=w1_bf[:, ko, q * QW : (q + 1) * QW],
                    start=(ko == 0),
                    stop=(ko == KO - 1),
                )
            nc.scalar.copy(out=re_sb[:b_loc, q * QW : (q + 1) * QW], in_=re_ps[:, :QW])
            for ho in range(q * QW // P, (q + 1) * QW // P):
                rtr = ps_rc.tile([P, TCHUNK], f32, tag="rc", name="rtr")
                nc.tensor.transpose(
                    rtr[:, :EP], re_sb[:, ho * P : (ho + 1) * P], id32
                )
                nc.vector.tensor_scalar(
                    out=reb_sb[:, ho, :],
                    in0=rtr[:, :b_loc],
                    scalar1=bias_sb[:, ho : ho + 1],
                    scalar2=None,
                    op0=mybir.AluOpType.add,
                )

        # ---------------- preamble loads (both HWDGE rings) ----------------
        # batch 0 needs w1+w2+its contexts (16.8 MB) before it can run free,
        # so spread the load: w1 on the scalar ring (feeds the entity path),
        # contexts-c0 + w2 + remaining contexts on the sync ring, all casts
        # on DVE ordered by consumption deadline.  The last two entity-path
        # quarters are deferred into chunk-0 slots so the PE never waits on
        # the late w1 quarters.
        for q in range(NQ):
            load_w_dma(w1_bf, w1_3d, q, nc.scalar)
        cast_w(w1_bf, 0)
        cast_w(w1_bf, 1)
        load_ctx(0, 0)
        load_w_dma(w2_bf, w2_3d, 0, nc.sync)
        load_w_dma(w2_bf, w2_3d, 1, nc.sync)
        emit_transpose(0, 0)
        load_w_dma(w2_bf, w2_3d, 2, nc.sync)
        load_w_dma(w2_bf, w2_3d, 3, nc.sync)
        cast_w(w2_bf, 0)
        cast_w(w1_bf, 2)
        cast_w(w2_bf, 1)
        cast_w(w1_bf, 3)
        cast_w(w2_bf, 2)
        cast_w(w2_bf, 3)
        load_ctx(0, 1)
        emit_transpose(0, 1)
        load_ctx(0, 2)
        emit_transpose(0, 2)
        load_ctx(0, 3)
        emit_transpose(0, 3)

        # weight staging is done being *emitted*; release its SBUF so the
        # stage-F / softmax pools (first used in the main loop) reuse it
        wst_ctx.close()
        th_pool = ctx.enter_context(tc.tile_pool(name="th", bufs=9))
        out_pool = ctx.enter_context(tc.tile_pool(name="outp", bufs=5))
        sm_pool = ctx.enter_context(tc.tile_pool(name="smx", bufs=2))

        emit_ent_transposes()
        emit_re_q(0)
        emit_re_q(1)

        # per-batch state shared between emission helpers
        state = {}

        def emit_matvec_block(b, T):
            # the 8 matvecs of a chunk run as 2 rounds of 4 packed into the
            # PE's four column-groups (tile_position): 4 concurrent M=1
            # matmuls cost ~one 512-cycle span instead of four.  Partial
            # score rows land on partitions 0/32/64/96 and are summed by the
            # k=4 score-transpose matmuls.
            st = state[b]
            sc4 = ps_sc.tile([P, TCHUNK], f32, tag="sc", name="sc_ps")
            st["sc"][T] = sc4
            for rnd in range(2):
                for j in range(4):
                    ho = rnd * 4 + j
                    nc.tensor.matmul(
                        sc4[32 * j : 32 * j + 1, :],
                        lhsT=v_bf[:, ho : ho + 1],
                        rhs=st["th"].pop((T, ho)),
                        start=(rnd == 0),
                        stop=(rnd == 1),
                        tile_position=(0, 32 * j),
                    )

        def emit_score_copy(b, T):
            # 4 partial rows PSUM -> SBUF fp16 (DVE has slack; these are off
            # the PE critical path thanks to the chunk stagger)
            st = state[b]
            swb = sm_pool.tile([P, TCHUNK], f16, tag="swb", bufs=3)
            nc.vector.memset(swb, 0.0)
            for j in range(4):
                nc.vector.tensor_copy(
                    out=swb[32 * j : 32 * j + 1, :],
                    in_=st["sc"][T][32 * j : 32 * j + 1, :],
                )
            st["swb"][T] = swb

        def emit_score_transpose(b, T):
            st = state[b]
            swb = st["swb"].pop(T)
            for j in range(TPC):
                cidx = T * TPC + j
                nc.tensor.matmul(
                    st["wt"][:, cidx : cidx + 1],
                    lhsT=swb[:, j * P : (j + 1) * P],
                    rhs=ones128_f16[:, :1],
                    start=(T == 0 and j == 0),
                    stop=(T == NC - 1 and j == TPC - 1),
                )

        def emit_softmax(b):
            st = state[b]
            wt = st["wt"]
            ew = sm_pool.tile([P, NT], f16, tag="ew")
            asum = sm_pool.tile([P, 1], f32, tag="asum")
            nc.scalar.activation(
                out=ew, in_=wt[:, :NT], func=AF.Exp, accum_out=asum
            )
            asum16 = sm_pool.tile([P, 1], f16, tag="asum16")
            nc.vector.tensor_copy(out=asum16, in_=asum)
            # cross-partition total, broadcast to every partition via ones^T @ asum
            nc.tensor.matmul(
                wt[:, NT : NT + 1], lhsT=ones128_f16, rhs=asum16, start=True, stop=True
            )
            rb = sm_pool.tile([P, 1], f32, tag="rb")
            nc.vector.reciprocal(out=rb, in_=wt[:, NT : NT + 1])
            wts = sm_pool.tile([P, NT], f32, tag="wts")
            nc.vector.tensor_scalar_mul(out=wts, in0=ew, scalar1=rb)
            st["wts"] = wts

        def emit_stage_f(b, c):
            # out = weight * contexts.  Mid-kernel both the multiplies and the
            # stores live on the otherwise-idle GPSIMD engine + SWDGE ring:
            # the pipeline self-paces on store completions without ever
            # FIFO-blocking tanhs (ACT), casts (DVE), or transposes (sync).
            # The tail batch splits multiplies across DVE/ACT for speed.
            st = state[b]
            wts = st["wts"]
            last = b == b_loc - 1
            for t in range(c * TPC, (c + 1) * TPC):
                src = xbf_tiles[(b, t // TPC)][:, t % TPC, :]
                ot = out_pool.tile([P, d], f32, tag="ot")
                if last and t % 2 == 1:
                    nc.scalar.activation(
                        out=ot, in_=src, func=AF.Copy, scale=wts[:, t : t + 1]
                    )
                else:
                    nc.vector.tensor_scalar_mul(
                        out=ot, in0=src, scalar1=wts[:, t : t + 1]
                    )
                nc.gpsimd.dma_start(out=out3[b * NT + t], in_=ot)
            xbf_tiles.pop((b, c))

        # ---------------- main loop over local batches ----------------
        for b in range(b_loc):
            state[b] = {"th": {}, "sc": {}, "swb": {}, "wt": None}
            state[b]["wt"] = ps_wt.tile([P, NT + 1], f32, tag="wt", name="wt_ps")

            for T in range(NC):
                # sync-ring FIFO discipline: ALL of this batch's transposes are
                # queued at the batch top, ahead of the next batch's context
                # loads, so a stalled context DMA can never head-of-line-block
                # a deadline-critical transpose.
                if T == 0 and b > 0:
                    for T2 in range(1, NC):
                        emit_transpose(b, T2)
                if b + 1 < b_loc:
                    if T == NC - 1:
                        emit_transpose(b + 1, 0)
                    load_ctx(b + 1, T)
                xt = xt_tiles.pop((b, T))

                for ho in range(HO):
                    rc = ps_rc.tile([P, TCHUNK], f32, tag="rc")
                    for ko in range(KO):
                        nc.tensor.matmul(
                            rc,
                            lhsT=w2_bf[:, ko, ho * P : (ho + 1) * P],
                            rhs=xt[:, :, ko, :],
                            start=(ko == 0),
                            stop=(ko == KO - 1),
                        )
                    th = th_pool.tile([P, TCHUNK], bf16, tag="th")
                    nc.scalar.activation(
                        out=th,
                        in_=rc,
                        func=AF.Tanh,
                        bias=reb_sb[:, ho, b : b + 1],
                        scale=1.0,
                    )
                    state[b]["th"][(T, ho)] = th

                    # deferred PE work, staggered one chunk so it never
                    # waits on ACT
                    if T >= 1:
                        if ho == 0:
                            emit_matvec_block(b, T - 1)
                            emit_score_copy(b, T - 1)
                        elif ho == 1:
                            emit_score_transpose(b, T - 1)
                    elif b >= 1:
                        # previous batch's tail rides this batch's first chunk
                        if ho == 0:
                            emit_matvec_block(b - 1, NC - 1)
                            emit_score_copy(b - 1, NC - 1)
                        elif ho == 1:
                            emit_score_transpose(b - 1, NC - 1)
                        elif ho == 2:
                            emit_softmax(b - 1)
                    else:
                        # b == 0, T == 0: finish the entity path once the
                        # late w1 quarters have streamed in
                        if ho == 1:
                            emit_re_q(2)
                        elif ho == 3:
                            emit_re_q(3)
                    if b >= 1 and ho == 3:
                        emit_stage_f(b - 1, T)
                        if T == NC - 1:
                            del state[b - 1]

        # tail: last batch's remaining score work + softmax + stage F
        bl = b_loc - 1
        emit_matvec_block(bl, NC - 1)
        emit_score_copy(bl, NC - 1)
        emit_score_transpose(bl, NC - 1)
        emit_softmax(bl)
        for c in range(NC):
            emit_stage_f(bl, c)


def build_module(b_loc=B_LOC, s=S, d=D, h=H):
    """Build and compile the Bacc module for one core (SPMD-replicated)."""
    import concourse.mybir as mybir
    import concourse.tile as tile
    from concourse import bacc

    f32 = mybir.dt.float32
    nc = bacc.Bacc("TRN2", target_bir_lowering=False, debug=False)

    ins = {
        "contexts": nc.dram_tensor("contexts", [b_loc * s, d], f32, kind="ExternalInput").ap(),
        "entities": nc.dram_tensor("entities", [b_loc, d], f32, kind="ExternalInput").ap(),
        "w1_w": nc.dram_tensor("w1_w", [d, h], f32, kind="ExternalInput").ap(),
        "w2_w": nc.dram_tensor("w2_w", [d, h], f32, kind="ExternalInput").ap(),
        "w1_b": nc.dram_tensor("w1_b", [h], f32, kind="ExternalInput").ap(),
        "w2_b": nc.dram_tensor("w2_b", [h], f32, kind="ExternalInput").ap(),
        "v_w": nc.dram_tensor("v_w", [h, 1], f32, kind="ExternalInput").ap(),
    }
    out_ap = nc.dram_tensor("out", [b_loc * s, d], f32, kind="ExternalOutput").ap()

    with tile.TileContext(nc) as tc:
        build_attention(tc, out_ap, ins, b_loc=b_loc, s=s, d=d, h=h)

    nc.compile()
    return nc


_NC_CACHE = {}


def _get_module():
    key = (B_LOC, S, D, H)
    if key not in _NC_CACHE:
        _NC_CACHE[key] = build_module(*key)
    return _NC_CACHE[key]


def make_in_maps(inputs):
    entities = np.ascontiguousarray(np.asarray(inputs["entities"], np.float32))
    contexts = np.ascontiguousarray(np.asarray(inputs["contexts"], np.float32))
    shared = {
        k: np.ascontiguousarray(np.asarray(inputs[k], np.float32))
        for k in ("w1_w", "w2_w", "w1_b", "w2_b", "v_w")
    }
    in_maps = []
    for c in range(N_CORES):
        in_maps.append(
            dict(
                entities=entities[c * B_LOC : (c + 1) * B_LOC],
                contexts=contexts[c * B_LOC : (c + 1) * B_LOC].reshape(B_LOC * S, D),
                **shared,
            )
        )
    return in_maps


def run(inputs, trace=False, **kwargs):
    """Run on all 8 cores; returns (full_output, BassKernelResults)."""
    from concourse.bass_utils import run_bass_kernel_spmd

    nc = _get_module()
    res = run_bass_kernel_spmd(
        nc, make_in_maps(inputs), core_ids=list(range(N_CORES)), trace=trace, **kwargs
    )
    out = np.concatenate(
        [res.results[c]["out"].reshape(B_LOC, S, D) for c in range(N_CORES)], axis=0
    )
    return out, res


def kernel(**inputs) -> np.ndarray:
    out, _ = run(inputs, trace=False)
    return out


# revision 68
# speedup vs baseline: 1.0914x; 1.0914x over previous
"""Trainium2 Bass kernel for nn_AttentionLayer (additive attention pooling).

reference math:
    re = entities @ w1_w + w1_b                  # [B, H]
    rc = contexts @ w2_w + w2_b                  # [B, S, H]
    scores = tanh(re[:,None,:] + rc) @ v_w + v_b # [B, S, 1]
    weights = softmax(scores, axis=1)
    out = weights * contexts                     # [B, S, D]

Sharding: data-parallel over B across 8 cores (4 batches/core), weights
replicated.  Inside each core: bf16 TensorEngine matmuls (f32 accumulate),
softmax in f32/f16.  v_b is dropped (softmax is shift-invariant).

Dataflow (per core), engineered so the TensorE 216ns/matmul stream never
stalls:
  - dual-ring preamble: w1 streams on the scalar HWDGE ring (feeding the
    entity path at the PE-program head, its last two quarters deferred into
    chunk-0 slots), contexts-c0 + w2 + remaining batch-0 contexts on the
    sync ring; casts on DVE ordered by consumption deadline.
  - per 512-token chunk: one xbar DMA transpose (sync ring) makes the
    d-major bf16 rhs; 64 accumulating bf16 matmuls produce rc; tanh(+re
    bias) on ACT; the 8 v-matvecs of a chunk run as one consecutive block
    one chunk later (entering/leaving the matvec costs a ~190ns LDW-pipeline
    bubble, and the stagger hides the tanh latency).
  - sync-ring FIFO discipline: all of a batch's transposes are queued ahead
    of the next batch's context loads so a stalled context DMA can never
    head-of-line-block a deadline-critical transpose.
  - chunk scores [1,512] are copied to fp16 and transposed token-major via
    4 tiny PE matmuls into a per-batch [128,16] PSUM tile; softmax runs
    full-width (exp+accum on ACT, cross-partition total broadcast via a
    ones-matmul, reciprocal on DVE) - no single-lane work.
  - out tiles = bf16 contexts * per-token weight (DVE multiply), stored on
    the otherwise-idle SWDGE (gpsimd) ring, spread across the next batch's
    chunk slots; each prior batch's tail (matvec block, score transpose,
    softmax, stage F) rides the next batch's first-chunk slots.

(An fp8e4m3 DoubleRow variant of the main matmul was built and passed
at rel-err 9.7e-3, but on this stack DoubleRow streams the two fp8 planes
sequentially - no throughput gain over bf16 - so bf16 is kept.)
"""

import sys

for _p in ("/opt/trn_rl_repo", "/root/.axon_site/_ro/trn_rl_repo"):
    if _p not in sys.path:
        sys.path.insert(0, _p)

import numpy as np

B, S, D, H = 32, 2048, 1024, 1024
N_CORES = 8
B_LOC = B // N_CORES          # batches per core
P = 128
TCHUNK = 512                  # tokens per chunk (moving free dim of main matmul)


def build_attention(tc, out_ap, ins, b_loc=B_LOC, s=S, d=D, h=H):
    """Emit the per-core kernel into TileContext `tc`.

    out_ap: DRAM AP [b_loc*s, d] f32
    ins: dict of DRAM APs: contexts [b_loc*s, d], entities [b_loc, d],
         w1_w [d, h], w2_w [d, h], w1_b [h], w2_b [h], v_w [h, 1]
    """
    from contextlib import ExitStack

    import concourse.mybir as mybir
    from concourse.masks import make_identity

    nc = tc.nc
    f32 = mybir.dt.float32
    bf16 = mybir.dt.bfloat16
    f16 = mybir.dt.float16
    AF = mybir.ActivationFunctionType

    KO = d // P                   # contraction k-tiles
    HO = h // P                   # h tiles
    NT = s // P                   # 128-token tiles per batch
    NC = s // TCHUNK              # chunks per batch
    TPC = TCHUNK // P             # token tiles per chunk
    QW = 256                      # h-chunk width for weight staging DMAs
    NQ = h // QW
    EP = 32                       # padded partition count for entity transposes
    assert d % P == 0 and h % P == 0 and s % TCHUNK == 0 and b_loc <= EP

    ctx3 = ins["contexts"].rearrange("(n p) dd -> n p dd", p=P)   # [b_loc*NT, P, d]
    out3 = out_ap.rearrange("(n p) dd -> n p dd", p=P)
    w1_3d = ins["w1_w"].rearrange("(ko p) hh -> p ko hh", p=P)
    w2_3d = ins["w2_w"].rearrange("(ko p) hh -> p ko hh", p=P)

    with ExitStack() as ctx:
        consts = ctx.enter_context(tc.tile_pool(name="consts", bufs=1))
        wpool = ctx.enter_context(tc.tile_pool(name="wpool", bufs=1))

        # ---------------- constants (tiny, sync ring) ----------------
        id32 = consts.tile([EP, EP], f32, tag="id32")
        make_identity(nc, id32)
        ones1_f16 = consts.tile([1, 1], f16, tag="ones1")
        nc.vector.memset(ones1_f16, 1.0)
        ones128_f16 = consts.tile([P, P], f16, tag="ones128")
        nc.vector.memset(ones128_f16, 1.0)

        ent_sb = consts.tile([EP, d], f32, tag="ent")
        nc.vector.memset(ent_sb, 0.0)
        nc.sync.dma_start(out=ent_sb[:b_loc, :], in_=ins["entities"][:, :])

        # biases and v load as contiguous [HO, P] rows (8 fat descriptors
        # instead of 1024 4-byte ones that would stall the ring for ~10us),
        # then are transposed to [P, HO] on the PE like the entity path
        b1_row = consts.tile([EP, P], f32, tag="b1r")
        b2_row = consts.tile([EP, P], f32, tag="b2r")
        v_row = consts.tile([EP, P], f32, tag="v_r")
        nc.vector.memset(b1_row, 0.0)
        nc.vector.memset(b2_row, 0.0)
        nc.vector.memset(v_row, 0.0)
        nc.sync.dma_start(
            out=b1_row[:HO, :], in_=ins["w1_b"].rearrange("(ho p) -> ho p", p=P)
        )
        nc.sync.dma_start(
            out=b2_row[:HO, :], in_=ins["w2_b"].rearrange("(ho p) -> ho p", p=P)
        )
        nc.sync.dma_start(
            out=v_row[:HO, :], in_=ins["v_w"].rearrange("(ho p) o -> ho (p o)", p=P)
        )
        bias_sb = consts.tile([P, HO], f32, tag="bias")

        # ---------------- weight / context tiles ----------------
        w1_bf = wpool.tile([P, KO, h], bf16, tag="w1bf")
        w2_bf = wpool.tile([P, KO, h], bf16, tag="w2bf")
        v_bf = consts.tile([P, HO], bf16, tag="v_bf")

        xbf_pool = ctx.enter_context(tc.tile_pool(name="xbf", bufs=8))
        xt_pool = ctx.enter_context(tc.tile_pool(name="xt", bufs=5))
        cin_pool = ctx.enter_context(tc.tile_pool(name="cin", bufs=3))
        # wst_pool is scoped: its SBUF is released before th/out/sm pools are
        # created, so the weight staging can afford 4 rotation buffers
        wst_ctx = ExitStack()
        wst_pool = wst_ctx.enter_context(tc.tile_pool(name="wst", bufs=4))

        wst_tiles = {}

        def load_w_dma(dst3, src3, q, eng):
            # HWDGE f32 load into a staging buffer; cast emitted separately
            # so the DMA order and cast order can differ
            wst = wst_pool.tile([P, KO, QW], f32, tag="wst")
            eng.dma_start(out=wst, in_=src3[:, :, q * QW : (q + 1) * QW])
            wst_tiles[(id(dst3), q)] = wst

        def cast_w(dst3, q):
            wst = wst_tiles.pop((id(dst3), q))
            nc.vector.tensor_copy(out=dst3[:, :, q * QW : (q + 1) * QW], in_=wst)

        xbf_tiles = {}
        xt_tiles = {}

        def load_ctx(b, c, eng=None):
            # per-chunk bf16 context tile, staged through two f32 half-chunk DMAs
            eng = eng or nc.sync
            xc = xbf_pool.tile([P, TPC, d], bf16, tag="xbf")
            r0 = b * NT + c * TPC
            for hf in range(2):
                cin = cin_pool.tile([P, 2, d], f32, tag="cin")
                eng.dma_start(
                    out=cin,
                    in_=ctx3[r0 + 2 * hf : r0 + 2 * hf + 2].rearrange(
                        "n p dd -> p n dd"
                    ),
                )
                nc.vector.tensor_copy(out=xc[:, 2 * hf : 2 * hf + 2, :], in_=cin)
            xbf_tiles[(b, c)] = xc

        def emit_transpose(b, T):
            xt = xt_pool.tile([P, TPC, KO, P], bf16, tag="xt", name="xt")
            nc.sync.dma_start(out=xt, in_=xbf_tiles[(b, T)], transpose=True)
            xt_tiles[(b, T)] = xt

        # ---------------- main-loop PSUM pools ----------------
        ps_rc = ctx.enter_context(tc.tile_pool(name="ps_rc", bufs=4, space="PSUM"))
        ps_sc = ctx.enter_context(tc.tile_pool(name="ps_sc", bufs=2, space="PSUM"))
        ps_wt = ctx.enter_context(tc.tile_pool(name="ps_wt", bufs=2, space="PSUM"))

        # ---------------- entity path: reb[:, ho, b] = (entities@w1 + b1+b2)^T ----
        reb_sb = consts.tile([P, HO, b_loc], f32, tag="reb")
        entT_bf = consts.tile([P, KO, b_loc], bf16, tag="entT")
        re_sb = consts.tile([EP, h], f32, tag="re_sb")

        # the tiny entity-path PSUM tiles borrow rc/sc bank slots (PSUM is
        # bank-granular and all 8 banks are budgeted for the main loop)
        def emit_ent_transposes():
            # ent transposes first: they only need ent_sb+id32 (~ready at
            # 10us); the bias/v transposes depend on DVE work queued behind
            # the big weight/context casts and would head-of-line-block the
            # PE if emitted first
            for ko in range(KO):
                etr = ps_rc.tile([P, TCHUNK], f32, tag="rc", name="etr")
                nc.tensor.transpose(
                    etr[:, :EP], ent_sb[:, ko * P : (ko + 1) * P], id32
                )
                nc.vector.tensor_copy(out=entT_bf[:, ko, :], in_=etr[:, :b_loc])
            badd_row = consts.tile([EP, P], f32, tag="badd")
            nc.vector.tensor_add(out=badd_row, in0=b1_row, in1=b2_row)
            btr = ps_rc.tile([P, TCHUNK], f32, tag="rc", name="btr")
            nc.tensor.transpose(btr[:, :EP], badd_row, id32)
            nc.vector.tensor_copy(out=bias_sb, in_=btr[:, :HO])
            vtr = ps_rc.tile([P, TCHUNK], f32, tag="rc", name="vtr")
            nc.tensor.transpose(vtr[:, :EP], v_row, id32)
            nc.vector.tensor_copy(out=v_bf, in_=vtr[:, :HO])
            nc.vector.memset(re_sb, 0.0)

        def emit_re_q(q):
            re_ps = ps_sc.tile([b_loc, TCHUNK], f32, tag="sc", name="re_ps")
            for ko in range(KO):
                nc.tensor.matmul(
                    re_ps[:, :QW],
                    lhsT=entT_bf[:, ko, :],
                    rhs=w1_bf[:, ko, q * QW : (q + 1) * QW],
                    start=(ko == 0),
                    stop=(ko == KO - 1),
                )
            nc.scalar.copy(out=re_sb[:b_loc, q * QW : (q + 1) * QW], in_=re_ps[:, :QW])
            for ho in range(q * QW // P, (q + 1) * QW // P):
                rtr = ps_rc.tile([P, TCHUNK], f32, tag="rc", name="rtr")
                nc.tensor.transpose(
                    rtr[:, :EP], re_sb[:, ho * P : (ho + 1) * P], id32
                )
                nc.vector.tensor_scalar(
                    out=reb_sb[:, ho, :],
                    in0=rtr[:, :b_loc],
                    scalar1=bias_sb[:, ho : ho + 1],
                    scalar2=None,
                    op0=mybir.AluOpType.add,
                )

        # ---------------- preamble loads (both HWDGE rings) ----------------
        # batch 0 needs w1+w2+its contexts (16.8 MB) before it can run free,
        # so spread the load: w1 on the scalar ring (feeds the entity path),
        # contexts-c0 + w2 + remaining contexts on the sync ring, all casts
        # on DVE ordered by consumption deadline.  The last two entity-path
        # quarters are deferred into chunk-0 slots so the PE never waits on
        # the late w1 quarters.
        for q in range(NQ):
            load_w_dma(w1_bf, w1_3d, q, nc.scalar)
        cast_w(w1_bf, 0)
        cast_w(w1_bf, 1)
        load_ctx(0, 0)
        load_w_dma(w2_bf, w2_3d, 0, nc.sync)
        load_w_dma(w2_bf, w2_3d, 1, nc.sync)
        emit_transpose(0, 0)
        load_w_dma(w2_bf, w2_3d, 2, nc.sync)
        load_w_dma(w2_bf, w2_3d, 3, nc.sync)
        cast_w(w2_bf, 0)
        cast_w(w1_bf, 2)
        cast_w(w2_bf, 1)
        cast_w(w1_bf, 3)
        cast_w(w2_bf, 2)
        cast_w(w2_bf, 3)
        load_ctx(0, 1)
        emit_transpose(0, 1)
        load_ctx(0, 2)
        emit_transpose(0, 2)
        load_ctx(0, 3)
        emit_transpose(0, 3)

        # weight staging is done being *emitted*; release its SBUF so the
        # stage-F / softmax pools (first used in the main loop) reuse it
        wst_ctx.close()
        th_pool = ctx.enter_context(tc.tile_pool(name="th", bufs=9))
        out_pool = ctx.enter_context(tc.tile_pool(name="outp", bufs=5))
        sm_pool = ctx.enter_context(tc.tile_pool(name="smx", bufs=2))

        emit_ent_transposes()
        emit_re_q(0)
        emit_re_q(1)

        # per-batch state shared between emission helpers
        state = {}

        def emit_matvec_block(b, T):
            # all 8 matvecs of a chunk back-to-back: entering/leaving the
            # matvec disturbs the main-MM LDW pipeline (~190ns bubble), so
            # pay it once per chunk rather than once per ho
            st = state[b]
            st["sc"][T] = ps_sc.tile([b_loc, TCHUNK], f32, tag="sc", name="sc_ps")[
                :1, :
            ]
            for ho in range(HO):
                nc.tensor.matmul(
                    st["sc"][T],
                    lhsT=v_bf[:, ho : ho + 1],
                    rhs=st["th"].pop((T, ho)),
                    start=(ho == 0),
                    stop=(ho == HO - 1),
                )

        def emit_score_copy(b, T):
            st = state[b]
            swb = sm_pool.tile([1, TCHUNK], f16, tag="swb", bufs=3)
            nc.scalar.copy(out=swb, in_=st["sc"][T])
            st["swb"][T] = swb

        def emit_score_transpose(b, T):
            st = state[b]
            swb = st["swb"].pop(T)
            for j in range(TPC):
                cidx = T * TPC + j
                nc.tensor.matmul(
                    st["wt"][:, cidx : cidx + 1],
                    lhsT=swb[:, j * P : (j + 1) * P],
                    rhs=ones1_f16,
                    start=(T == 0 and j == 0),
                    stop=(T == NC - 1 and j == TPC - 1),
                )

        def emit_softmax(b):
            st = state[b]
            wt = st["wt"]
            ew = sm_pool.tile([P, NT], f16, tag="ew")
            asum = sm_pool.tile([P, 1], f32, tag="asum")
            nc.scalar.activation(
                out=ew, in_=wt[:, :NT], func=AF.Exp, accum_out=asum
            )
            asum16 = sm_pool.tile([P, 1], f16, tag="asum16")
            nc.vector.tensor_copy(out=asum16, in_=asum)
            # cross-partition total, broadcast to every partition via ones^T @ asum
            nc.tensor.matmul(
                wt[:, NT : NT + 1], lhsT=ones128_f16, rhs=asum16, start=True, stop=True
            )
            rb = sm_pool.tile([P, 1], f32, tag="rb")
            nc.vector.reciprocal(out=rb, in_=wt[:, NT : NT + 1])
            wts = sm_pool.tile([P, NT], f32, tag="wts")
            nc.vector.tensor_scalar_mul(out=wts, in0=ew, scalar1=rb)
            st["wts"] = wts

        def emit_stage_f(b, c):
            # out = weight * contexts.  Mid-kernel both the multiplies and the
            # stores live on the otherwise-idle GPSIMD engine + SWDGE ring:
            # the pipeline self-paces on store completions without ever
            # FIFO-blocking tanhs (ACT), casts (DVE), or transposes (sync).
            # The tail batch splits multiplies across DVE/ACT for speed.
            st = state[b]
            wts = st["wts"]
            last = b == b_loc - 1
            for t in range(c * TPC, (c + 1) * TPC):
                src = xbf_tiles[(b, t // TPC)][:, t % TPC, :]
                ot = out_pool.tile([P, d], f32, tag="ot")
                if last and t % 2 == 1:
                    nc.scalar.activation(
                        out=ot, in_=src, func=AF.Copy, scale=wts[:, t : t + 1]
                    )
                else:
                    nc.vector.tensor_scalar_mul(
                        out=ot, in0=src, scalar1=wts[:, t : t + 1]
                    )
                nc.gpsimd.dma_start(out=out3[b * NT + t], in_=ot)
            xbf_tiles.pop((b, c))

        # ---------------- main loop over local batches ----------------
        for b in range(b_loc):
            state[b] = {"th": {}, "sc": {}, "swb": {}, "wt": None}
            state[b]["wt"] = ps_wt.tile([P, NT + 1], f32, tag="wt", name="wt_ps")

            for T in range(NC):
                # sync-ring FIFO discipline: ALL of this batch's transposes are
                # queued at the batch top, ahead of the next batch's context
                # loads, so a stalled context DMA can never head-of-line-block
                # a deadline-critical transpose.
                if T == 0 and b > 0:
                    for T2 in range(1, NC):
                        emit_transpose(b, T2)
                if b + 1 < b_loc:
                    if T == NC - 1:
                        emit_transpose(b + 1, 0)
                    load_ctx(b + 1, T)
                xt = xt_tiles.pop((b, T))

                for ho in range(HO):
                    rc = ps_rc.tile([P, TCHUNK], f32, tag="rc")
                    for ko in range(KO):
                        nc.tensor.matmul(
                            rc,
                            lhsT=w2_bf[:, ko, ho * P : (ho + 1) * P],
                            rhs=xt[:, :, ko, :],
                            start=(ko == 0),
                            stop=(ko == KO - 1),
                        )
                    th = th_pool.tile([P, TCHUNK], bf16, tag="th")
                    nc.scalar.activation(
                        out=th,
                        in_=rc,
                        func=AF.Tanh,
                        bias=reb_sb[:, ho, b : b + 1],
                        scale=1.0,
                    )
                    state[b]["th"][(T, ho)] = th

                    # deferred PE work, staggered one chunk so it never
                    # waits on ACT
                    if T >= 1:
                        if ho == 0:
                            emit_matvec_block(b, T - 1)
                            emit_score_copy(b, T - 1)
                        elif ho == 1:
                            emit_score_transpose(b, T - 1)
                    elif b >= 1:
                        # previous batch's tail rides this batch's first chunk
                        if ho == 0:
                            emit_matvec_block(b - 1, NC - 1)
                            emit_score_copy(b - 1, NC - 1)
                        elif ho == 1:
                            emit_score_transpose(b - 1, NC - 1)
                        elif ho == 2:
                            emit_softmax(b - 1)
                    else:
                        # b == 0, T == 0: finish the entity path once the
                        # late w1 quarters have streamed in
                        if ho == 1:
                            emit_re_q(2)
                        elif ho == 3:
                            emit_re_q(3)
                    if b >= 1 and ho == 3:
                        emit_stage_f(b - 1, T)
                        if T == NC - 1:
                            del state[b - 1]

        # tail: last batch's remaining score work + softmax + stage F
        bl = b_loc - 1
        emit_matvec_block(bl, NC - 1)
        emit_score_copy(bl, NC - 1)
        emit_score_transpose(bl, NC - 1)
        emit_softmax(bl)
        for c in range(NC):
            emit_stage_f(bl, c)


def build_module(b_loc=B_LOC, s=S, d=D, h=H):
    """Build and compile the Bacc module for one core (SPMD-replicated)."""
    import concourse.mybir as mybir
    import concourse.tile as tile
    from concourse import bacc

    f32 = mybir.dt.float32
    nc = bacc.Bacc("TRN2", target_bir_lowering=False, debug=False)

    ins = {
        "contexts": nc.dram_tensor("contexts", [b_loc * s, d], f32, kind="ExternalInput").ap(),
        "entities": nc.dram_tensor("entities", [b_loc, d], f32, kind="ExternalInput").ap(),
        "w1_w": nc.dram_tensor("w1_w", [d, h], f32, kind="ExternalInput").ap(),
        "w2_w": nc.dram_tensor("w2_w", [d, h], f32, kind="ExternalInput").ap(),
        "w1_b": nc.dram_tensor("w1_b", [h], f32, kind="ExternalInput").ap(),
        "w2_b": nc.dram_tensor("w2_b", [h], f32, kind="ExternalInput").ap(),
        "v_w": nc.dram_tensor("v_w", [h, 1], f32, kind="ExternalInput").ap(),
    }
    out_ap = nc.dram_tensor("out", [b_loc * s, d], f32, kind="ExternalOutput").ap()

    with tile.TileContext(nc) as tc:
        build_attention(tc, out_ap, ins, b_loc=b_loc, s=s, d=d, h=h)

    nc.compile()
    return nc


_NC_CACHE = {}


def _get_module():
    key = (B_LOC, S, D, H)
    if key not in _NC_CACHE:
        _NC_CACHE[key] = build_module(*key)
    return _NC_CACHE[key]


def make_in_maps(inputs):
    entities = np.ascontiguousarray(np.asarray(inputs["entities"], np.float32))
    contexts = np.ascontiguousarray(np.asarray(inputs["contexts"], np.float32))
    shared = {
        k: np.ascontiguousarray(np.asarray(inputs[k], np.float32))
        for k in ("w1_w", "w2_w", "w1_b", "w2_b", "v_w")
    }
    in_maps = []
    for c in range(N_CORES):
        in_maps.append(
            dict(
                entities=entities[c * B_LOC : (c + 1) * B_LOC],
                contexts=contexts[c * B_LOC : (c + 1) * B_LOC].reshape(B_LOC * S, D),
                **shared,
            )
        )
    return in_maps


def run(inputs, trace=False, **kwargs):
    """Run on all 8 cores; returns (full_output, BassKernelResults)."""
    from concourse.bass_utils import run_bass_kernel_spmd

    nc = _get_module()
    res = run_bass_kernel_spmd(
        nc, make_in_maps(inputs), core_ids=list(range(N_CORES)), trace=trace, **kwargs
    )
    out = np.concatenate(
        [res.results[c]["out"].reshape(B_LOC, S, D) for c in range(N_CORES)], axis=0
    )
    return out, res


def kernel(**inputs) -> np.ndarray:
    out, _ = run(inputs, trace=False)
    return out
